# revision 1
# baseline (speedup 1.0000x reference)
"""Trainium2 Bass kernel for nn_Cross_attention_2 (sparse_attention).

Math (B=1, C=32, D=36, H=W=48, P=9):
  xc = conv1x1(x, W_img, b_img)            # per-voxel channel mix
  v  = unfold(xc)                          # (C, L=1024, 81) non-overlapping 9x9 patches
  px = LeakyReLU(v @ (W2@W1)^T + bias)     # the two Linears collapse to A = W2@W1
  att[c] = px[c] @ py[c]^T / 81            # (C, 1024, 1024)

Sharding: channels C=32 split across 8 cores (4 each). Params replicated
(per-core slices precomputed on host). Each core reads full x, y.

Per-core device pipeline (fp32 data, fp32r PE mode; all matmul outputs at
PSUM partition base 0 — fp32r codegen requires it):
  conv:      3 accumulating zero-padded block-diag matmuls (K=128/128/32)
             -> xc_sb (37, 4, 2304): rows kd*4+o, row 36 = 1.0 (bias row)
  transform: unfold folded into strided rhs APs; 9 kw-accumulation passes per
             output tile; combined weight TM includes channel select + A + bias
  att:       pxT/pyT kept as (81, 1024); out tiles (128, 512) per matmul
"""

import sys

sys.path.insert(0, "/opt/trn_rl_repo")

import contextlib
import os

import numpy as np

import concourse.bass as bass  # noqa: F401
import concourse.tile as tile
from concourse import bacc, mybir
from concourse.bass_utils import run_bass_kernel_spmd

P = 9
P2 = 81
C = 32
D = 36
HWF = 2304
ND = 4  # pd blocks (D/9)
L = 1024
N_CORES = 8
CPC = 4  # channels per core

F32 = mybir.dt.float32
F32R = mybir.dt.float32r

_CACHE = {}
last_results = None  # BassKernelResults of the most recent run (for test.py)

_HW_CHUNKS = [(0, 512), (512, 512), (1024, 512), (1536, 512), (2048, 256)]
_KD_PASSES = [(0, 4), (4, 4), (8, 1)]  # (kd0, nkd) conv passes


def _build():
    if "nc" in _CACHE:
        return _CACHE["nc"]

    nc = bacc.Bacc("TRN2", target_bir_lowering=False, debug=False,
                   num_devices=N_CORES)
    x_d = nc.dram_tensor("x", (C, D, HWF), F32R, kind="ExternalInput").ap()
    y_d = nc.dram_tensor("y", (C, D, HWF), F32R, kind="ExternalInput").ap()
    # wblk: (128, 216) = conv lhsT for (t in 2) x (pass i in 3), 36 cols each
    wblk_d = nc.dram_tensor("wblk", (128, 216), F32R, kind="ExternalInput").ap()
    # tm: (37, 2*4*9*81) combined transform weights in SBUF layout
    tm_d = nc.dram_tensor("tm", (37, 2 * CPC * P * P2), F32R,
                          kind="ExternalInput").ap()
    ones_d = nc.dram_tensor("ones", (1, ND * HWF), F32R,
                            kind="ExternalInput").ap()
    att_d = nc.dram_tensor("att", (CPC, L, L), F32, kind="ExternalOutput").ap()

    with tile.TileContext(nc) as tc:
        with contextlib.ExitStack() as ctx:
            consts = ctx.enter_context(tc.tile_pool(name="consts", bufs=1))
            xbp = ctx.enter_context(tc.tile_pool(name="xb", bufs=3))
            xbp2 = ctx.enter_context(tc.tile_pool(name="xb2", bufs=1))
            tmpp = ctx.enter_context(tc.tile_pool(name="tmp", bufs=2))
            outp = ctx.enter_context(tc.tile_pool(name="outp", bufs=2))
            cps = ctx.enter_context(tc.tile_pool(name="cps", bufs=2, space="PSUM"))
            tps = ctx.enter_context(tc.tile_pool(name="tps", bufs=2, space="PSUM"))
            aps = ctx.enter_context(tc.tile_pool(name="aps", bufs=3, space="PSUM"))

            wb_sb = consts.tile([128, 216], F32R, tag="wb")
            nc.sync.dma_start(out=wb_sb[:, :], in_=wblk_d[:, :])
            tm_sb = consts.tile([37, 2 * CPC * P * P2], F32R, tag="tm")
            nc.sync.dma_start(out=tm_sb[:, :], in_=tm_d[:, :])
            tm_v = tm_sb.rearrange("p (t c kw j) -> p t c kw j", t=2, c=CPC, kw=P)

            xc_sb = []
            px_sb = []
            for t in range(2):
                xt = consts.tile([37, ND, HWF], F32R, tag=f"xc{t}")
                nc.sync.dma_start(
                    out=xt[36:37, :, :],
                    in_=ones_d.rearrange("p (d h) -> p d h", d=ND))
                xc_sb.append(xt)
                px_sb.append([consts.tile([P2, L], F32R, tag=f"px{t}{c}",
                                          name=f"px{t}{c}")
                              for c in range(CPC)])

            for t in range(2):
                src = x_d if t == 0 else y_d
                for pd in range(ND):
                    xbs = []
                    for i, (kd0, nkd) in enumerate(_KD_PASSES):
                        kp = 32 * nkd
                        pool = xbp if nkd == 4 else xbp2
                        xb = pool.tile([kp, HWF], F32R, tag=f"xb{min(i, 1)}",
                                       name=f"xb{min(i, 1)}")
                        rows = src[:, pd * P + kd0: pd * P + kd0 + nkd, :]
                        nc.sync.dma_start(out=xb[:, :],
                                          in_=rows.transpose([1, 0, 2]))
                        xbs.append(xb)
                    for h0, hn in _HW_CHUNKS:
                        ps = cps.tile([36, 512], F32, tag="cps")
                        for i, (kd0, nkd) in enumerate(_KD_PASSES):
                            kp = 32 * nkd
                            lhs = wb_sb[0:kp,
                                        (t * 3 + i) * 36: (t * 3 + i + 1) * 36]
                            nc.tensor.matmul(
                                ps[:, :hn], lhs, xbs[i][:, h0: h0 + hn],
                                start=(i == 0), stop=(i == 2))
                        dst = xc_sb[t][0:36, pd, h0: h0 + hn]
                        if (h0 // 512) % 2 == 0:
                            nc.vector.tensor_copy(out=dst, in_=ps[:, :hn])
                        else:
                            nc.scalar.copy(out=dst, in_=ps[:, :hn])

                # transform: z = sum_kw TM[t,c,kw].T @ xc[:, :, kw::9]
                for c in range(CPC):
                    for ch in range(2):  # l-chunks of 512 (pd pairs)
                        zp = tps.tile([P2, 512], F32, tag="tps")
                        for kw in range(P):
                            rhs = xc_sb[t][:, 2 * ch: 2 * ch + 2, kw:HWF:P]
                            nc.tensor.matmul(
                                zp[:, :], tm_v[:, t, c, kw, :], rhs,
                                start=(kw == 0), stop=(kw == P - 1))
                        # LeakyReLU(z) = max(0.2*z, z)
                        zm = tmpp.tile([P2, 512], F32, tag="zm")
                        nc.scalar.mul(zm[:, :], zp[:, :], 0.2)
                        nc.vector.tensor_tensor(
                            out=px_sb[t][c][:, ch * 512: ch * 512 + 512],
                            in0=zp[:, :], in1=zm[:, :],
                            op=mybir.AluOpType.max)

            # att[c] = pxT[c].T @ pyT[c]
            for c in range(CPC):
                for m in range(8):  # l1 chunks of 128
                    ob = outp.tile([128, L], F32, tag="ob")
                    for nch in range(2):  # l2 chunks of 512
                        ap_ = aps.tile([128, 512], F32, tag="aps")
                        nc.tensor.matmul(
                            ap_[:, :],
                            px_sb[0][c][:, m * 128: m * 128 + 128],
                            px_sb[1][c][:, nch * 512: nch * 512 + 512],
                            start=True, stop=True)
                        dst = ob[:, nch * 512: nch * 512 + 512]
                        if nch % 2 == 0:
                            nc.vector.tensor_copy(out=dst, in_=ap_[:, :])
                        else:
                            nc.scalar.copy(out=dst, in_=ap_[:, :])
                    nc.sync.dma_start(
                        out=att_d[c, m * 128: m * 128 + 128, :], in_=ob[:, :])

    nc.compile()
    _CACHE["nc"] = nc
    return nc


def _host_prep(x, y, W_img, b_img, W_fea, b_fea, W1, W2):
    """Build per-core wblk / tm arrays. Returns in_maps list."""
    x = np.ascontiguousarray(np.asarray(x, np.float32).reshape(C, D, HWF))
    y = np.ascontiguousarray(np.asarray(y, np.float32).reshape(C, D, HWF))
    W_img = np.asarray(W_img, np.float32)
    b_img = np.asarray(b_img, np.float32)
    W_fea = np.asarray(W_fea, np.float32)
    b_fea = np.asarray(b_fea, np.float32)
    A = np.asarray(W2, np.float32) @ np.asarray(W1, np.float32)  # (81, 81)
    rowsum = A.sum(axis=1)  # (81,)
    ones = np.ones((1, ND * HWF), np.float32)

    in_maps = []
    for r in range(N_CORES):
        Wl = [W_img[r * CPC:(r + 1) * CPC, :], W_fea[r * CPC:(r + 1) * CPC, :]]
        bl = [b_img[r * CPC:(r + 1) * CPC], b_fea[r * CPC:(r + 1) * CPC]]

        # conv lhsT: wblk[kd_l*32+c', (t*3+i)*36 + kd*4+o] = W_t[o, c']
        #            with kd = kd0_i + kd_l
        wblk = np.zeros((128, 216), np.float32)
        for t in range(2):
            for i, (kd0, nkd) in enumerate(_KD_PASSES):
                for kd_l in range(nkd):
                    kd = kd0 + kd_l
                    rows = slice(kd_l * 32, kd_l * 32 + 32)
                    for o in range(CPC):
                        col = (t * 3 + i) * 36 + kd * 4 + o
                        wblk[rows, col] = Wl[t][o, :]

        # tm[p, t, c, kw, j]; p = kd*4 + o, row 36 = bias (kw=0 only)
        tm = np.zeros((37, 2, CPC, P, P2), np.float32)
        At = np.stack([A / P2, A])                 # x-side carries the 1/81
        bias = np.stack([np.outer(bl[0], rowsum) / P2,
                         np.outer(bl[1], rowsum)])  # (2, 4, 81)
        for kd in range(P):
            for o in range(CPC):
                p = kd * 4 + o
                # tm[p, t, o, kw, j] = At[t, j, kd*9+kw]
                tm[p, :, o, :, :] = At[:, :, kd * P:(kd + 1) * P].transpose(0, 2, 1)
        tm[36, :, :, 0, :] = bias
        tm = tm.reshape(37, 2 * CPC * P * P2)

        in_maps.append({"x": x, "y": y, "wblk": wblk,
                        "tm": np.ascontiguousarray(tm), "ones": ones})
    return in_maps


def kernel(**inputs):
    global last_results
    nc = _build()
    in_maps = _host_prep(**inputs)
    trace = bool(os.environ.get("KERNEL_TRACE"))
    res = run_bass_kernel_spmd(nc, in_maps, core_ids=list(range(N_CORES)),
                               trace=trace)
    last_results = res
    att = np.stack([res.results[r]["att"] for r in range(N_CORES)])
    return att.reshape(1, C, L, L)



# revision 5
# speedup vs baseline: 1.9065x; 1.9065x over previous
"""Trainium2 Bass kernel for nn_Cross_attention_2 (sparse_attention).

Math (B=1, C=32, D=36, H=W=48, P=9):
  xc = conv1x1(x, W_img, b_img)            # per-voxel channel mix (bias deferred)
  v  = unfold(xc)                          # (C, L=1024, 81) non-overlapping 9x9 patches
  px = LeakyReLU(v @ (W2@W1)^T + bias)     # the two Linears collapse to A = W2@W1
  att[c] = px[c] @ py[c]^T / 81            # (C, 1024, 1024)

Sharding: channels C=32 split across 8 cores (4 each). Params replicated
(per-core slices precomputed on host). Each core reads full x, y.

Per-core device pipeline, all data fp16 (fp32 PSUM accumulation):
  load:      x, y cast to fp16 on host; per-(t,pd) loads with c-major outer AP
             so descriptors spread across the 16 SDMA engines
  conv:      3 accumulating matmuls (K=128/128/32) -> psum (36,512) chunks
             -> xc3 rows 0-35 (rows kd*4+o); conv bias NOT added here
  shift:     SBUF->SBUF DMAs replicate xc3 into rows 36-71 (cols shifted -3)
             and 72-107 (shifted -6) so one transform pass covers 3 kw values
  transform: 3 matmul passes (K=108) per output tile; epilogue is one scalar
             Lrelu activation: out = LeakyReLU(psum/9 + b_c*rowsum(A)/9)
             (1/9 per side => att carries the full 1/81)
  att:       pxT (81,1024) per (t,c); out tiles (128,512); fp16 store,
             host casts back to fp32
"""

import sys

sys.path.insert(0, "/opt/trn_rl_repo")

import contextlib
import os

import numpy as np

import concourse.bass as bass  # noqa: F401
import concourse.tile as tile
from concourse import bacc, mybir
from concourse.bass_utils import run_bass_kernel_spmd

P = 9
P2 = 81
C = 32
D = 36
HWF = 2304
ND = 4  # pd blocks (D/9)
L = 1024
N_CORES = 8
CPC = 4  # channels per core

F32 = mybir.dt.float32
F16 = mybir.dt.float16

_CACHE = {}
last_results = None  # BassKernelResults of the most recent run (for test.py)

_HW_CHUNKS = [(0, 512), (512, 512), (1024, 512), (1536, 512), (2048, 256)]


def _build():
    if "nc" in _CACHE:
        return _CACHE["nc"]

    nc = bacc.Bacc("TRN2", target_bir_lowering=False, debug=False,
                   num_devices=N_CORES)
    x_d = nc.dram_tensor("x", (C, D, HWF), F16, kind="ExternalInput").ap()
    y_d = nc.dram_tensor("y", (C, D, HWF), F16, kind="ExternalInput").ap()
    # wblk: conv lhsT, rows p = c*4+kd_l (i<2) or p = c (i=2);
    # col blocks (t*3+i)*36 + kd*4+o
    wblk_d = nc.dram_tensor("wblk", (128, 216), F16, kind="ExternalInput").ap()
    # tm3: (108, t*4c*3p*81j) transform weights, rows g*36 + kd*4 + o
    tm_d = nc.dram_tensor("tm", (108, 2 * CPC * 3 * P2), F16,
                          kind="ExternalInput").ap()
    ones_d = nc.dram_tensor("ones", (1, ND * HWF), F16,
                            kind="ExternalInput").ap()
    att_d = nc.dram_tensor("att", (CPC, L, L), F16, kind="ExternalOutput").ap()

    with tile.TileContext(nc) as tc:
        with contextlib.ExitStack() as ctx:
            consts = ctx.enter_context(tc.tile_pool(name="consts", bufs=1))
            xbp = ctx.enter_context(tc.tile_pool(name="xb", bufs=4))
            xb8p = ctx.enter_context(tc.tile_pool(name="xb8", bufs=2))
            tmpp = ctx.enter_context(tc.tile_pool(name="tmp", bufs=2))
            outp = ctx.enter_context(tc.tile_pool(name="outp", bufs=3))
            cps = ctx.enter_context(tc.tile_pool(name="cps", bufs=2, space="PSUM"))
            tps = ctx.enter_context(tc.tile_pool(name="tps", bufs=2, space="PSUM"))
            aps = ctx.enter_context(tc.tile_pool(name="aps", bufs=3, space="PSUM"))

            wb_sb = consts.tile([128, 216], F16, tag="wb")
            nc.sync.dma_start(out=wb_sb[:, :], in_=wblk_d[:, :])
            tm_sb = consts.tile([108, 2 * CPC * 3 * P2], F16, tag="tm")
            nc.sync.dma_start(out=tm_sb[:, :], in_=tm_d[:, :])
            tm_v = tm_sb.rearrange("p (t c k j) -> p t c k j", t=2, c=CPC, k=3)

            xc3 = []
            px_sb = []
            for t in range(2):
                xc3.append(consts.tile([108, ND, HWF], F16, tag=f"xc{t}",
                                       name=f"xc{t}"))
                px_sb.append([consts.tile([P2, L], F16, tag=f"px{t}{c}",
                                          name=f"px{t}{c}")
                              for c in range(CPC)])

            ncopy = 0
            for t in range(2):
                src = x_d if t == 0 else y_d
                xb8 = xb8p.tile([33, ND, HWF], F16, tag="xb8")
                nc.sync.dma_start(out=xb8[0:32, :, :], in_=src[:, 8::9, :])
                nc.sync.dma_start(
                    out=xb8[32:33, :, :],
                    in_=ones_d.rearrange("p (d h) -> p d h", d=ND))
                for pd in range(ND):
                    xb = xbp.tile([128, 2, HWF], F16, tag="xb")
                    for i in range(2):
                        d0 = 9 * pd + 4 * i
                        nc.sync.dma_start(out=xb[:, i, :],
                                          in_=src[:, d0: d0 + 4, :])
                    for h0, hn in _HW_CHUNKS:
                        ps = cps.tile([36, 512], F32, tag="cps")
                        for i in range(3):
                            if i < 2:
                                lhs = wb_sb[0:128,
                                            (t * 3 + i) * 36:(t * 3 + i + 1) * 36]
                                rhs = xb[:, i, h0: h0 + hn]
                            else:
                                lhs = wb_sb[0:33,
                                            (t * 3 + 2) * 36:(t * 3 + 3) * 36]
                                rhs = xb8[0:33, pd, h0: h0 + hn]
                            nc.tensor.matmul(ps[:, :hn], lhs, rhs,
                                             start=(i == 0), stop=(i == 2))
                        dst = xc3[t][0:36, pd, h0: h0 + hn]
                        if ncopy % 2 == 0:
                            nc.vector.tensor_copy(out=dst, in_=ps[:, :hn])
                        else:
                            nc.scalar.copy(out=dst, in_=ps[:, :hn])
                        ncopy += 1
                    # replicate into kw-shifted row groups (rows 36-71, 72-107)
                    for g in (1, 2):
                        s = 3 * g
                        nc.sync.dma_start(
                            out=xc3[t][36 * g: 36 * g + 36, pd, 0: HWF - s],
                            in_=xc3[t][0:36, pd, s:HWF])

            # transform: z = sum_p TM[t,c,p].T @ xc3[:, pd-pair, p::9]
            for t in range(2):
                for c in range(CPC):
                    for ch in range(2):  # l-chunks of 512 (pd pairs)
                        zp = tps.tile([P2, 512], F32, tag="tps")
                        for p in range(3):
                            rhs = xc3[t][0:108, 2 * ch: 2 * ch + 2, p:HWF:P]
                            nc.tensor.matmul(
                                zp[:, :], tm_v[:, t, c, p, :], rhs,
                                start=(p == 0), stop=(p == 2))
                        zm = tmpp.tile([P2, 512], F32, tag="zm")
                        nc.scalar.mul(zm[:, :], zp[:, :], 0.2)
                        nc.vector.tensor_tensor(
                            out=px_sb[t][c][:, ch * 512: ch * 512 + 512],
                            in0=zp[:, :], in1=zm[:, :],
                            op=mybir.AluOpType.max)

            # att[c] = pxT[c].T @ pyT[c]
            for c in range(CPC):
                for m in range(8):  # l1 chunks of 128
                    ob = outp.tile([128, L], F16, tag="ob")
                    for nch in range(2):  # l2 chunks of 512
                        ap_ = aps.tile([128, 512], F32, tag="aps")
                        nc.tensor.matmul(
                            ap_[:, :],
                            px_sb[0][c][:, m * 128: m * 128 + 128],
                            px_sb[1][c][:, nch * 512: nch * 512 + 512],
                            start=True, stop=True)
                        dst = ob[:, nch * 512: nch * 512 + 512]
                        if nch % 2 == 0:
                            nc.vector.tensor_scalar_mul(dst, ap_[:, :],
                                                        1.0 / P2)
                        else:
                            nc.scalar.mul(dst, ap_[:, :], 1.0 / P2)
                    nc.sync.dma_start(
                        out=att_d[c, m * 128: m * 128 + 128, :], in_=ob[:, :])

    nc.compile()
    _CACHE["nc"] = nc
    return nc


def _host_prep(x, y, W_img, b_img, W_fea, b_fea, W1, W2):
    """Build per-core wblk / tm / bias arrays. Returns in_maps list."""
    x = np.ascontiguousarray(
        np.asarray(x, np.float32).reshape(C, D, HWF).astype(np.float16))
    y = np.ascontiguousarray(
        np.asarray(y, np.float32).reshape(C, D, HWF).astype(np.float16))
    W_img = np.asarray(W_img, np.float32)
    b_img = np.asarray(b_img, np.float32)
    W_fea = np.asarray(W_fea, np.float32)
    b_fea = np.asarray(b_fea, np.float32)
    A = np.asarray(W2, np.float32) @ np.asarray(W1, np.float32)  # (81, 81)

    in_maps = []
    for r in range(N_CORES):
        Wl = [W_img[r * CPC:(r + 1) * CPC, :], W_fea[r * CPC:(r + 1) * CPC, :]]
        bl = [b_img[r * CPC:(r + 1) * CPC], b_fea[r * CPC:(r + 1) * CPC]]

        # conv lhsT: passes i<2 rows p=c*4+kd_l (kd=4i+kd_l), pass 2 rows p=c
        wblk = np.zeros((128, 216), np.float32)
        for t in range(2):
            for i in range(2):
                for kd_l in range(4):
                    kd = 4 * i + kd_l
                    for o in range(CPC):
                        col = (t * 3 + i) * 36 + kd * 4 + o
                        wblk[kd_l::4, col] = Wl[t][o, :]  # rows c*4+kd_l
            for o in range(CPC):
                col = (t * 3 + 2) * 36 + 8 * 4 + o
                wblk[0:32, col] = Wl[t][o, :]
            for kd in range(P):
                for o in range(CPC):
                    wblk[32, (t * 3 + 2) * 36 + kd * 4 + o] = bl[t][o]

        # tm3[p, t, c, pass, j]; p = g*36 + kd*4 + o, kw = pass + 3g
        tm = np.zeros((108, 2, CPC, 3, P2), np.float32)
        for g in range(3):
            for kd in range(P):
                for o in range(CPC):
                    row = g * 36 + kd * 4 + o
                    for pp in range(3):
                        kw = pp + 3 * g
                        tm[row, :, o, pp, :] = A[:, kd * P + kw]
        tm = tm.reshape(108, 2 * CPC * 3 * P2)

        in_maps.append({
            "x": x, "y": y,
            "wblk": wblk.astype(np.float16),
            "tm": np.ascontiguousarray(tm.astype(np.float16)),
            "ones": np.ones((1, ND * HWF), np.float16),
        })
    return in_maps


def kernel(**inputs):
    global last_results
    nc = _build()
    in_maps = _host_prep(**inputs)
    trace = bool(os.environ.get("KERNEL_TRACE"))
    res = run_bass_kernel_spmd(nc, in_maps, core_ids=list(range(N_CORES)),
                               trace=trace)
    last_results = res
    att = np.stack([res.results[r]["att"] for r in range(N_CORES)])
    return att.reshape(1, C, L, L).astype(np.float32)


# revision 6
# speedup vs baseline: 2.4552x; 1.2878x over previous
"""Trainium2 Bass kernel for nn_Cross_attention_2 (sparse_attention).

Math (B=1, C=32, D=36, H=W=48, P=9):
  xc = conv1x1(x, W_img, b_img)            # per-voxel channel mix (bias deferred)
  v  = unfold(xc)                          # (C, L=1024, 81) non-overlapping 9x9 patches
  px = LeakyReLU(v @ (W2@W1)^T + bias)     # the two Linears collapse to A = W2@W1
  att[c] = px[c] @ py[c]^T / 81            # (C, 1024, 1024)

Sharding: channels C=32 split across 8 cores (4 each). Params replicated
(per-core slices precomputed on host). Each core reads full x, y.

Per-core device pipeline, all data fp16 (fp32 PSUM accumulation):
  load:      x, y cast to fp16 on host; per-(t,pd) loads with c-major outer AP
             so descriptors spread across the 16 SDMA engines
  conv:      3 accumulating matmuls (K=128/128/32) -> psum (36,512) chunks
             -> xc3 rows 0-35 (rows kd*4+o); conv bias NOT added here
  shift:     SBUF->SBUF DMAs replicate xc3 into rows 36-71 (cols shifted -3)
             and 72-107 (shifted -6) so one transform pass covers 3 kw values
  transform: 3 matmul passes (K=108) per output tile; epilogue is one scalar
             Lrelu activation: out = LeakyReLU(psum/9 + b_c*rowsum(A)/9)
             (1/9 per side => att carries the full 1/81)
  att:       pxT (81,1024) per (t,c); out tiles (128,512); fp16 store,
             host casts back to fp32
"""

import sys

sys.path.insert(0, "/opt/trn_rl_repo")

import contextlib
import os

import numpy as np

import concourse.bass as bass  # noqa: F401
import concourse.tile as tile
from concourse import bacc, mybir
from concourse.bass_utils import run_bass_kernel_spmd

P = 9
P2 = 81
C = 32
D = 36
HWF = 2304
ND = 4  # pd blocks (D/9)
L = 1024
N_CORES = 8
CPC = 4  # channels per core

F32 = mybir.dt.float32
F32R = mybir.dt.float32r
F16 = mybir.dt.float16

_CACHE = {}
last_results = None  # BassKernelResults of the most recent run (for test.py)

_HW_CHUNKS = [(0, 512), (512, 512), (1024, 512), (1536, 512), (2048, 256)]


def _build():
    if "nc" in _CACHE:
        return _CACHE["nc"]

    nc = bacc.Bacc("TRN2", target_bir_lowering=False, debug=False,
                   num_devices=N_CORES)
    x_d = nc.dram_tensor("x", (C, D, HWF), F16, kind="ExternalInput").ap()
    y_d = nc.dram_tensor("y", (C, D, HWF), F16, kind="ExternalInput").ap()
    # wblk: conv lhsT, rows p = c*4+kd_l (i<2) or p = c (i=2);
    # col blocks (t*3+i)*36 + kd*4+o
    wblk_d = nc.dram_tensor("wblk", (128, 216), F16, kind="ExternalInput").ap()
    # tm3: (108, t*4c*3p*81j) transform weights, rows g*36 + kd*4 + o
    tm_d = nc.dram_tensor("tm", (108, 2 * CPC * 3 * P2), F32R,
                          kind="ExternalInput").ap()
    ones_d = nc.dram_tensor("ones", (1, ND * HWF), F16,
                            kind="ExternalInput").ap()
    att_d = nc.dram_tensor("att", (CPC, L, L), F16, kind="ExternalOutput").ap()

    with tile.TileContext(nc) as tc:
        with contextlib.ExitStack() as ctx:
            consts = ctx.enter_context(tc.tile_pool(name="consts", bufs=1))
            xbp = ctx.enter_context(tc.tile_pool(name="xb", bufs=6))
            xb8p = ctx.enter_context(tc.tile_pool(name="xb8", bufs=2))
            tmpp = ctx.enter_context(tc.tile_pool(name="tmp", bufs=2))
            outp = ctx.enter_context(tc.tile_pool(name="outp", bufs=3))
            cps = ctx.enter_context(tc.tile_pool(name="cps", bufs=2, space="PSUM"))
            tps = ctx.enter_context(tc.tile_pool(name="tps", bufs=2, space="PSUM"))
            aps = ctx.enter_context(tc.tile_pool(name="aps", bufs=4, space="PSUM"))

            wb_sb = consts.tile([128, 216], F16, tag="wb")
            nc.sync.dma_start(out=wb_sb[:, :], in_=wblk_d[:, :])
            tm_sb = consts.tile([108, 2 * CPC * 3 * P2], F32R, tag="tm")
            nc.sync.dma_start(out=tm_sb[:, :], in_=tm_d[:, :])
            tm_v = tm_sb.rearrange("p (t c k j) -> p t c k j", t=2, c=CPC, k=3)

            xc3 = []
            px_sb = []
            for t in range(2):
                xc3.append(consts.tile([108, ND, HWF], F32R, tag=f"xc{t}",
                                       name=f"xc{t}"))
                px_sb.append([consts.tile([P2, L], F16, tag=f"px{t}{c}",
                                          name=f"px{t}{c}")
                              for c in range(CPC)])

            ncopy = 0
            for t in range(2):
                src = x_d if t == 0 else y_d
                xb8 = xb8p.tile([33, ND, HWF], F16, tag="xb8")
                xbs = []
                for pd in range(ND):
                    xb = xbp.tile([128, 2, HWF], F16, tag="xb")
                    for i in range(2):
                        d0 = 9 * pd + 4 * i
                        nc.sync.dma_start(out=xb[:, i, :],
                                          in_=src[:, d0: d0 + 4, :])
                    xbs.append(xb)
                    if pd == 0:
                        nc.sync.dma_start(out=xb8[0:32, :, :],
                                          in_=src[:, 8::9, :])
                        nc.sync.dma_start(
                            out=xb8[32:33, :, :],
                            in_=ones_d.rearrange("p (d h) -> p d h", d=ND))
                for pd in range(ND):
                    xb = xbs[pd]
                    for h0, hn in _HW_CHUNKS:
                        ps = cps.tile([36, 512], F32, tag="cps")
                        for i in range(3):
                            if i < 2:
                                lhs = wb_sb[0:128,
                                            (t * 3 + i) * 36:(t * 3 + i + 1) * 36]
                                rhs = xb[:, i, h0: h0 + hn]
                            else:
                                lhs = wb_sb[0:33,
                                            (t * 3 + 2) * 36:(t * 3 + 3) * 36]
                                rhs = xb8[0:33, pd, h0: h0 + hn]
                            nc.tensor.matmul(ps[:, :hn], lhs, rhs,
                                             start=(i == 0), stop=(i == 2))
                        dst = xc3[t][0:36, pd, h0: h0 + hn]
                        if ncopy % 2 == 0:
                            nc.vector.tensor_copy(out=dst, in_=ps[:, :hn])
                        else:
                            nc.scalar.copy(out=dst, in_=ps[:, :hn])
                        ncopy += 1
                    # replicate into kw-shifted row groups (rows 36-71, 72-107)
                    # on the SWDGE queue so it can't head-of-line-block loads
                    for g in (1, 2):
                        s = 3 * g
                        nc.gpsimd.dma_start(
                            out=xc3[t][36 * g: 36 * g + 36, pd, 0: HWF - s],
                            in_=xc3[t][0:36, pd, s:HWF])

            # per channel: transform both t, then att (so att stores
            # overlap the next channel's transform)
            for c in range(CPC):
                # transform: z = sum_p TM[t,c,p].T @ xc3[:, pd-pair, p::9]
                for t in range(2):
                    for ch in range(2):  # l-chunks of 512 (pd pairs)
                        zp = tps.tile([P2, 512], F32, tag="tps")
                        for p in range(3):
                            rhs = xc3[t][0:108, 2 * ch: 2 * ch + 2, p:HWF:P]
                            nc.tensor.matmul(
                                zp[:, :], tm_v[:, t, c, p, :], rhs,
                                start=(p == 0), stop=(p == 2))
                        zm = tmpp.tile([P2, 512], F32, tag="zm")
                        nc.scalar.mul(zm[:, :], zp[:, :], 0.2)
                        nc.vector.tensor_tensor(
                            out=px_sb[t][c][:, ch * 512: ch * 512 + 512],
                            in0=zp[:, :], in1=zm[:, :],
                            op=mybir.AluOpType.max)

                # att[c] = pxT[c].T @ pyT[c]
                for m in range(8):  # l1 chunks of 128
                    ob = outp.tile([128, L], F16, tag="ob")
                    for nch in range(2):  # l2 chunks of 512
                        ap_ = aps.tile([128, 512], F32, tag="aps")
                        nc.tensor.matmul(
                            ap_[:, :],
                            px_sb[0][c][:, m * 128: m * 128 + 128],
                            px_sb[1][c][:, nch * 512: nch * 512 + 512],
                            start=True, stop=True)
                        dst = ob[:, nch * 512: nch * 512 + 512]
                        if nch % 2 == 0:
                            nc.vector.tensor_scalar_mul(dst, ap_[:, :],
                                                        1.0 / P2)
                        else:
                            nc.scalar.mul(dst, ap_[:, :], 1.0 / P2)
                    nc.sync.dma_start(
                        out=att_d[c, m * 128: m * 128 + 128, :], in_=ob[:, :])

    nc.compile()
    _CACHE["nc"] = nc
    return nc


def _host_prep(x, y, W_img, b_img, W_fea, b_fea, W1, W2):
    """Build per-core wblk / tm / bias arrays. Returns in_maps list."""
    x = np.ascontiguousarray(
        np.asarray(x, np.float32).reshape(C, D, HWF).astype(np.float16))
    y = np.ascontiguousarray(
        np.asarray(y, np.float32).reshape(C, D, HWF).astype(np.float16))
    W_img = np.asarray(W_img, np.float32)
    b_img = np.asarray(b_img, np.float32)
    W_fea = np.asarray(W_fea, np.float32)
    b_fea = np.asarray(b_fea, np.float32)
    A = np.asarray(W2, np.float32) @ np.asarray(W1, np.float32)  # (81, 81)

    in_maps = []
    for r in range(N_CORES):
        Wl = [W_img[r * CPC:(r + 1) * CPC, :], W_fea[r * CPC:(r + 1) * CPC, :]]
        bl = [b_img[r * CPC:(r + 1) * CPC], b_fea[r * CPC:(r + 1) * CPC]]

        # conv lhsT: passes i<2 rows p=c*4+kd_l (kd=4i+kd_l), pass 2 rows p=c
        wblk = np.zeros((128, 216), np.float32)
        for t in range(2):
            for i in range(2):
                for kd_l in range(4):
                    kd = 4 * i + kd_l
                    for o in range(CPC):
                        col = (t * 3 + i) * 36 + kd * 4 + o
                        wblk[kd_l::4, col] = Wl[t][o, :]  # rows c*4+kd_l
            for o in range(CPC):
                col = (t * 3 + 2) * 36 + 8 * 4 + o
                wblk[0:32, col] = Wl[t][o, :]
            for kd in range(P):
                for o in range(CPC):
                    wblk[32, (t * 3 + 2) * 36 + kd * 4 + o] = bl[t][o]

        # tm3[p, t, c, pass, j]; p = g*36 + kd*4 + o, kw = pass + 3g
        tm = np.zeros((108, 2, CPC, 3, P2), np.float32)
        for g in range(3):
            for kd in range(P):
                for o in range(CPC):
                    row = g * 36 + kd * 4 + o
                    for pp in range(3):
                        kw = pp + 3 * g
                        tm[row, :, o, pp, :] = A[:, kd * P + kw]
        tm = tm.reshape(108, 2 * CPC * 3 * P2)

        in_maps.append({
            "x": x, "y": y,
            "wblk": wblk.astype(np.float16),
            "tm": np.ascontiguousarray(tm),
            "ones": np.ones((1, ND * HWF), np.float16),
        })
    return in_maps


def kernel(**inputs):
    global last_results
    nc = _build()
    in_maps = _host_prep(**inputs)
    trace = bool(os.environ.get("KERNEL_TRACE"))
    res = run_bass_kernel_spmd(nc, in_maps, core_ids=list(range(N_CORES)),
                               trace=trace)
    last_results = res
    att = np.stack([res.results[r]["att"] for r in range(N_CORES)])
    return att.reshape(1, C, L, L).astype(np.float32)


# revision 7
# speedup vs baseline: 2.4753x; 1.0082x over previous
"""Trainium2 Bass kernel for nn_Cross_attention_2 (sparse_attention).

Math (B=1, C=32, D=36, H=W=48, P=9):
  xc = conv1x1(x, W_img, b_img)            # per-voxel channel mix (bias deferred)
  v  = unfold(xc)                          # (C, L=1024, 81) non-overlapping 9x9 patches
  px = LeakyReLU(v @ (W2@W1)^T + bias)     # the two Linears collapse to A = W2@W1
  att[c] = px[c] @ py[c]^T / 81            # (C, 1024, 1024)

Sharding: channels C=32 split across 8 cores (4 each). Params replicated
(per-core slices precomputed on host). Each core reads full x, y.

Per-core device pipeline, all data fp16 (fp32 PSUM accumulation):
  load:      x, y cast to fp16 on host; per-(t,pd) loads with c-major outer AP
             so descriptors spread across the 16 SDMA engines
  conv:      3 accumulating matmuls (K=128/128/32) -> psum (36,512) chunks
             -> xc3 rows 0-35 (rows kd*4+o); conv bias NOT added here
  shift:     SBUF->SBUF DMAs replicate xc3 into rows 36-71 (cols shifted -3)
             and 72-107 (shifted -6) so one transform pass covers 3 kw values
  transform: 3 matmul passes (K=108) per output tile; epilogue is one scalar
             Lrelu activation: out = LeakyReLU(psum/9 + b_c*rowsum(A)/9)
             (1/9 per side => att carries the full 1/81)
  att:       pxT (81,1024) per (t,c); out tiles (128,512); fp16 store,
             host casts back to fp32
"""

import sys

sys.path.insert(0, "/opt/trn_rl_repo")

import contextlib
import os

import numpy as np

import concourse.bass as bass  # noqa: F401
import concourse.tile as tile
from concourse import bacc, mybir
from concourse.bass_utils import run_bass_kernel_spmd

P = 9
P2 = 81
C = 32
D = 36
HWF = 2304
ND = 4  # pd blocks (D/9)
L = 1024
N_CORES = 8
CPC = 4  # channels per core

F32 = mybir.dt.float32
F32R = mybir.dt.float32r
F16 = mybir.dt.float16

_CACHE = {}
last_results = None  # BassKernelResults of the most recent run (for test.py)

_HW_CHUNKS = [(0, 512), (512, 512), (1024, 512), (1536, 512), (2048, 256)]


def _build():
    if "nc" in _CACHE:
        return _CACHE["nc"]

    nc = bacc.Bacc("TRN2", target_bir_lowering=False, debug=False,
                   num_devices=N_CORES)
    x_d = nc.dram_tensor("x", (C, D, HWF), F16, kind="ExternalInput").ap()
    y_d = nc.dram_tensor("y", (C, D, HWF), F16, kind="ExternalInput").ap()
    # wblk: conv lhsT, rows p = c*4+kd_l (i<2) or p = c (i=2);
    # col blocks (t*3+i)*36 + kd*4+o
    wblk_d = nc.dram_tensor("wblk", (128, 216), F16, kind="ExternalInput").ap()
    # tm3: (108, t*4c*3p*81j) transform weights, rows g*36 + kd*4 + o
    tm_d = nc.dram_tensor("tm", (108, 2 * CPC * 3 * P2), F32R,
                          kind="ExternalInput").ap()
    ones_d = nc.dram_tensor("ones", (1, ND * HWF), F16,
                            kind="ExternalInput").ap()
    att_d = nc.dram_tensor("att", (CPC, L, L), F16, kind="ExternalOutput").ap()

    with tile.TileContext(nc) as tc:
        with contextlib.ExitStack() as ctx:
            consts = ctx.enter_context(tc.tile_pool(name="consts", bufs=1))
            xbp = ctx.enter_context(tc.tile_pool(name="xb", bufs=6))
            xb8p = ctx.enter_context(tc.tile_pool(name="xb8", bufs=2))
            tmpp = ctx.enter_context(tc.tile_pool(name="tmp", bufs=2))
            outp = ctx.enter_context(tc.tile_pool(name="outp", bufs=3))
            cps = ctx.enter_context(tc.tile_pool(name="cps", bufs=2, space="PSUM"))
            tps = ctx.enter_context(tc.tile_pool(name="tps", bufs=2, space="PSUM"))
            aps = ctx.enter_context(tc.tile_pool(name="aps", bufs=4, space="PSUM"))

            wb_sb = consts.tile([128, 216], F16, tag="wb")
            nc.scalar.dma_start(out=wb_sb[:, :], in_=wblk_d[:, :])
            tm_sb = consts.tile([108, 2 * CPC * 3 * P2], F32R, tag="tm")
            nc.scalar.dma_start(out=tm_sb[:, :], in_=tm_d[:, :])
            tm_v = tm_sb.rearrange("p (t c k j) -> p t c k j", t=2, c=CPC, k=3)

            xc3 = []
            px_sb = []
            for t in range(2):
                xc3.append(consts.tile([108, ND, HWF], F32R, tag=f"xc{t}",
                                       name=f"xc{t}"))
                px_sb.append([consts.tile([P2, L], F16, tag=f"px{t}{c}",
                                          name=f"px{t}{c}")
                              for c in range(CPC)])

            ncopy = 0
            for t in range(2):
                src = x_d if t == 0 else y_d
                xb8 = xb8p.tile([33, ND, HWF], F16, tag="xb8")
                xbs = []
                for pd in range(ND):
                    xb = xbp.tile([128, 2, HWF], F16, tag="xb")
                    for i in range(2):
                        d0 = 9 * pd + 4 * i
                        nc.sync.dma_start(out=xb[:, i, :],
                                          in_=src[:, d0: d0 + 4, :])
                    xbs.append(xb)
                    if pd == 0:
                        nc.sync.dma_start(out=xb8[0:32, :, :],
                                          in_=src[:, 8::9, :])
                        nc.scalar.dma_start(
                            out=xb8[32:33, :, :],
                            in_=ones_d.rearrange("p (d h) -> p d h", d=ND))
                for pd in range(ND):
                    xb = xbs[pd]
                    for h0, hn in _HW_CHUNKS:
                        ps = cps.tile([36, 512], F32, tag="cps")
                        for i in range(3):
                            if i < 2:
                                lhs = wb_sb[0:128,
                                            (t * 3 + i) * 36:(t * 3 + i + 1) * 36]
                                rhs = xb[:, i, h0: h0 + hn]
                            else:
                                lhs = wb_sb[0:33,
                                            (t * 3 + 2) * 36:(t * 3 + 3) * 36]
                                rhs = xb8[0:33, pd, h0: h0 + hn]
                            nc.tensor.matmul(ps[:, :hn], lhs, rhs,
                                             start=(i == 0), stop=(i == 2))
                        dst = xc3[t][0:36, pd, h0: h0 + hn]
                        if ncopy % 2 == 0:
                            nc.vector.tensor_copy(out=dst, in_=ps[:, :hn])
                        else:
                            nc.scalar.copy(out=dst, in_=ps[:, :hn])
                        ncopy += 1
                    # replicate into kw-shifted row groups (rows 36-71, 72-107)
                    # on the SWDGE queue so it can't head-of-line-block loads
                    for g in (1, 2):
                        s = 3 * g
                        nc.gpsimd.dma_start(
                            out=xc3[t][36 * g: 36 * g + 36, pd, 0: HWF - s],
                            in_=xc3[t][0:36, pd, s:HWF])

            # transform helper: z = sum_p TM[t,c,p].T @ xc3[:, pd-pair, p::9]
            def transform(t, c):
                for ch in range(2):  # l-chunks of 512 (pd pairs)
                    zp = tps.tile([P2, 512], F32, tag="tps")
                    for p in range(3):
                        rhs = xc3[t][0:108, 2 * ch: 2 * ch + 2, p:HWF:P]
                        nc.tensor.matmul(
                            zp[:, :], tm_v[:, t, c, p, :], rhs,
                            start=(p == 0), stop=(p == 2))
                    zm = tmpp.tile([P2, 512], F32, tag="zm")
                    nc.scalar.mul(zm[:, :], zp[:, :], 0.2)
                    nc.vector.tensor_tensor(
                        out=px_sb[t][c][:, ch * 512: ch * 512 + 512],
                        in0=zp[:, :], in1=zm[:, :],
                        op=mybir.AluOpType.max)

            # t0 transforms first (t1's last shift DMAs land meanwhile);
            # then per channel: t1 transform + att, so att stores overlap
            # the next channel's transform
            for c in range(CPC):
                transform(0, c)
            for c in range(CPC):
                transform(1, c)
                # att[c] = pxT[c].T @ pyT[c]
                for m in range(8):  # l1 chunks of 128
                    ob = outp.tile([128, L], F16, tag="ob")
                    for nch in range(2):  # l2 chunks of 512
                        ap_ = aps.tile([128, 512], F32, tag="aps")
                        nc.tensor.matmul(
                            ap_[:, :],
                            px_sb[0][c][:, m * 128: m * 128 + 128],
                            px_sb[1][c][:, nch * 512: nch * 512 + 512],
                            start=True, stop=True)
                        dst = ob[:, nch * 512: nch * 512 + 512]
                        if nch % 2 == 0:
                            nc.vector.tensor_scalar_mul(dst, ap_[:, :],
                                                        1.0 / P2)
                        else:
                            nc.scalar.mul(dst, ap_[:, :], 1.0 / P2)
                    nc.sync.dma_start(
                        out=att_d[c, m * 128: m * 128 + 128, :], in_=ob[:, :])

    nc.compile()
    _CACHE["nc"] = nc
    return nc


def _host_prep(x, y, W_img, b_img, W_fea, b_fea, W1, W2):
    """Build per-core wblk / tm / bias arrays. Returns in_maps list."""
    x = np.ascontiguousarray(
        np.asarray(x, np.float32).reshape(C, D, HWF).astype(np.float16))
    y = np.ascontiguousarray(
        np.asarray(y, np.float32).reshape(C, D, HWF).astype(np.float16))
    W_img = np.asarray(W_img, np.float32)
    b_img = np.asarray(b_img, np.float32)
    W_fea = np.asarray(W_fea, np.float32)
    b_fea = np.asarray(b_fea, np.float32)
    A = np.asarray(W2, np.float32) @ np.asarray(W1, np.float32)  # (81, 81)

    in_maps = []
    for r in range(N_CORES):
        Wl = [W_img[r * CPC:(r + 1) * CPC, :], W_fea[r * CPC:(r + 1) * CPC, :]]
        bl = [b_img[r * CPC:(r + 1) * CPC], b_fea[r * CPC:(r + 1) * CPC]]

        # conv lhsT: passes i<2 rows p=c*4+kd_l (kd=4i+kd_l), pass 2 rows p=c
        wblk = np.zeros((128, 216), np.float32)
        for t in range(2):
            for i in range(2):
                for kd_l in range(4):
                    kd = 4 * i + kd_l
                    for o in range(CPC):
                        col = (t * 3 + i) * 36 + kd * 4 + o
                        wblk[kd_l::4, col] = Wl[t][o, :]  # rows c*4+kd_l
            for o in range(CPC):
                col = (t * 3 + 2) * 36 + 8 * 4 + o
                wblk[0:32, col] = Wl[t][o, :]
            for kd in range(P):
                for o in range(CPC):
                    wblk[32, (t * 3 + 2) * 36 + kd * 4 + o] = bl[t][o]

        # tm3[p, t, c, pass, j]; p = g*36 + kd*4 + o, kw = pass + 3g
        tm = np.zeros((108, 2, CPC, 3, P2), np.float32)
        for g in range(3):
            for kd in range(P):
                for o in range(CPC):
                    row = g * 36 + kd * 4 + o
                    for pp in range(3):
                        kw = pp + 3 * g
                        tm[row, :, o, pp, :] = A[:, kd * P + kw]
        tm = tm.reshape(108, 2 * CPC * 3 * P2)

        in_maps.append({
            "x": x, "y": y,
            "wblk": wblk.astype(np.float16),
            "tm": np.ascontiguousarray(tm),
            "ones": np.ones((1, ND * HWF), np.float16),
        })
    return in_maps


def kernel(**inputs):
    global last_results
    nc = _build()
    in_maps = _host_prep(**inputs)
    trace = bool(os.environ.get("KERNEL_TRACE"))
    res = run_bass_kernel_spmd(nc, in_maps, core_ids=list(range(N_CORES)),
                               trace=trace)
    last_results = res
    att = np.stack([res.results[r]["att"] for r in range(N_CORES)])
    return att.reshape(1, C, L, L).astype(np.float32)
